# revision 30
# baseline (speedup 1.0000x reference)
"""Multi-head attention (B=2, S=2048, D=1024, H=16) on 8 trn2 NeuronCores.

Sharding: 2-way batch x 4-way head-group tensor parallel. Core c handles
batch c//4 and heads 4*(c%4) .. 4*(c%4)+3 (a 256-wide feature slice of the
q/k/v projections, and the matching row-slice of the out projection). Each
core emits a full-size [2048, 1024] bf16 partial of the output; the host sums
the 4 partials per batch (f32) and adds the output bias.

v3 data flow (per core):
  - Activations arrive feature-major bf16 ([D, S], host-pretransposed), all
    weights bf16. All matmuls bf16 with f32 PSUM.
  - Q/K feature-major: QT/KT [dq, t] bf16. Per score k-tile, BOTH heads of
    a pair run as [64]-contraction matmuls into ONE psum tile (adjacent
    banks) at PE row-halves 0/64 (tile_position), back-to-back with equal
    deps so the hardware co-issues them concurrently into the two array
    halves — a score pair costs one 512-col PE slot instead of two.
  - V token-major bf16 with 64 ones-columns appended, so attn.V also yields
    the softmax denominator on psum rows 64..127.
  - exp() on ScalarE reads each [128, 2, 512] scores psum tile once and
    writes the merged head-pair PT [128, 2, kt, 512] bf16. The Scalar queue
    carries ONLY the exp stream (plus tail-only outproj casts).
  - Emission is software-pipelined: scores(qc0) ladder with per-chunk K
    projection; V/Q projections and the previous chunk's out-projection are
    interleaved between scores/attnV so PE stays fed while ACT runs exp.
"""

import ml_dtypes
import numpy as np

import concourse.bacc as bacc
import concourse.mybir as mybir
import concourse.tile as tile
from concourse.bass_interp import get_hw_module
from concourse.bass_utils import run_bass_kernel_spmd

# problem constants (hardcoded; must match the reference)
B = 2
S = 2048
D = 1024
NH = 16
DH = 64
SCALE = DH ** -0.5

# sharding
N_CORES = 8
HG = 4                # heads per core
F = HG * DH           # 256 projected features per core
CH = 512              # token chunk
NCH = S // CH         # 4 chunks
P = 128
FT = D // P           # 8 feature tiles
MT = F // P           # 2 projected-feature tiles
KT = S // P           # 16 key-token tiles
KG = KT // 2          # 8 k-tile pairs (psum/exp groups)

f32 = mybir.dt.float32
bf16 = mybir.dt.bfloat16
EXP = mybir.ActivationFunctionType.Exp


def _emit(ctx, nc, tc, aps):
    xqT, xkT, xvT, wqT, wkT, wvT, woT, bq2, bk2, bv1, out = aps

    consts = ctx.enter_context(tc.tile_pool(name="consts", bufs=1))
    persist = ctx.enter_context(tc.tile_pool(name="persist", bufs=1))

    # biases + out-proj weight on the gpsimd DMA queue (scalar queue is
    # reserved for the exp stream; sync queue carries the big x/w stream)
    bq_sb = consts.tile([P, MT], f32)
    bk_sb = consts.tile([P, MT], f32)
    nc.gpsimd.dma_start(out=bq_sb, in_=bq2)
    nc.gpsimd.dma_start(out=bk_sb, in_=bk2)
    bv_sb = consts.tile([P, F], f32)
    nc.gpsimd.dma_start(out=bv_sb, in_=bv1.unsqueeze(0).to_broadcast((P, F)))
    wo_sb = consts.tile([P, MT, D], bf16)

    # persistent activations
    QT_sb = persist.tile([P, MT, NCH, CH], bf16)   # [dq%128, dq//128, qc, q]
    KT_sb = persist.tile([P, MT, NCH, CH], bf16)
    # V'' layout: [k%128, k//128, h, dv | 64 ones columns]
    V_sb = persist.tile([P, KT, HG, P], bf16)

    # 3 w slots: w_v's DMA trigger sits on the scalar queue AHEAD of the
    # exp stream — with 2 slots it would wait on w_k's readers (WAW) and
    # stall every exp behind it.
    w_pool = ctx.enter_context(tc.tile_pool(name="w_pool", bufs=3))
    xT_pool = ctx.enter_context(tc.tile_pool(name="xT_pool", bufs=8))
    # PSUM budget (8 banks): scores 2x2 (double-buffered co-issue pairs),
    # attn.V 2x1 (the two trailer chains of one stream), proj/outproj 2x1.
    ps_proj = ctx.enter_context(tc.tile_pool(name="ps_proj", bufs=2, space="PSUM"))
    ps_s = ctx.enter_context(tc.tile_pool(name="ps_s", bufs=2, space="PSUM"))
    ps_av = ctx.enter_context(tc.tile_pool(name="ps_av", bufs=2, space="PSUM"))
    pt_pool = ctx.enter_context(tc.tile_pool(name="pt_pool", bufs=2))
    ot_pool = ctx.enter_context(tc.tile_pool(name="ot_pool", bufs=1))
    o_stage = ctx.enter_context(tc.tile_pool(name="o_stage", bufs=3))
    rc_pool = ctx.enter_context(tc.tile_pool(name="rc_pool", bufs=1))

    OT_sb = ot_pool.tile([P, MT, NCH, CH], bf16)

    # PE warmup: chained dummy matmuls spin the PE through the initial DMA
    # wait so the clock is at full p-state when real work arrives. ~12 cold
    # matmuls cover the HAM's 3.4us busy window without overshooting the
    # lead-in DMA (first projection can start ~10us in).
    warm_sb = consts.tile([P, CH], bf16)
    nc.vector.memset(warm_sb, 0.0)
    ps_warm = ps_proj.tile([P, CH], f32, tag="proj")
    for i in range(10):
        nc.tensor.matmul(
            ps_warm, warm_sb[:, 0:P], warm_sb,
            start=(i == 0), stop=(i == 9),
        )

    # ---- phase-A building blocks -------------------------------------
    def load_w(wT_ap, engine=None):
        w_sb = w_pool.tile([P, FT, F], bf16, tag="w")
        (engine or nc.sync).dma_start(out=w_sb, in_=wT_ap)
        return w_sb

    def load_x(xT_ap, c):
        xT = xT_pool.tile([P, FT, CH], bf16, tag="xT")
        nc.sync.dma_start(out=xT, in_=xT_ap[:, c])
        return xT

    def proj_qk_m(w_sb, xT, c, m, is_q):
        """One m-half (128 projected features) of a Q/K chunk projection."""
        b_sb = bq_sb if is_q else bk_sb
        dst = QT_sb if is_q else KT_sb
        ps = ps_proj.tile([P, CH], f32, tag="proj")
        for ft in range(FT):
            nc.tensor.matmul(
                ps, w_sb[:, ft, m * P:(m + 1) * P], xT[:, ft, :],
                start=(ft == 0), stop=(ft == FT - 1),
            )
        nc.vector.tensor_scalar_add(dst[:, m, c, :], ps, b_sb[:, m:m + 1])

    def proj_v_t4(w_sb, xT, c, t4):
        """One 128-token block of a V chunk projection."""
        ps = ps_proj.tile([P, F], f32, tag="proj")
        for ft in range(FT):
            nc.tensor.matmul(
                ps, xT[:, ft, t4 * P:(t4 + 1) * P], w_sb[:, ft, :],
                start=(ft == 0), stop=(ft == FT - 1),
            )
        kt = c * (CH // P) + t4
        nc.vector.tensor_add(
            V_sb[:, kt, :, 0:DH],
            ps.rearrange("p (h d) -> p h d", h=HG),
            bv_sb.rearrange("p (h d) -> p h d", h=HG),
        )

    # ---- phase-B building blocks -------------------------------------
    def new_pt():
        # merged head-pair PT: [key%128, h%2, kt, q]
        pt = pt_pool.tile([P, 2, KT, CH], bf16, tag="PT")
        return pt

    def scores_unit(qc, mh, pt, kt):
        """One score k-tile for head pair (2*mh, 2*mh+1).

        Both heads' [64]-contraction matmuls land in ONE psum tile
        (adjacent banks) with identical deps, emitted back-to-back at PE
        row-halves 0/64 (tile_position) so the hardware co-issues them
        into the two array halves concurrently. One ACT drains the pair
        into the merged PT.
        """
        ps = ps_s.tile([P, 2, CH], f32, tag="s")
        ktc, kto = kt // 4, (kt % 4) * P
        for par in (0, 1):
            p0 = par * DH
            nc.tensor.matmul(
                ps[:, par, :],
                KT_sb[p0:p0 + DH, mh, ktc, kto:kto + P],
                QT_sb[p0:p0 + DH, mh, qc, :],
                start=True, stop=True,
                tile_position=(p0, 0),
            )
        nc.scalar.activation(
            out=pt[:, :, kt, :], in_=ps, func=EXP, scale=SCALE
        )

    def make_av(qc, h, scalar_copy=False):
        """Incremental attn.V chain for head h: call emit(kt) per k-tile."""
        mh, par = divmod(h, 2)
        st = {}

        def emit(kt):
            if kt < 0 or kt >= KT:
                return
            if kt == 0:
                st['po'] = ps_av.tile(
                    [P, CH], f32, tag="o", name=f"po_{qc}_{h}"
                )
            po = st['po']
            nc.tensor.matmul(
                po, V_sb[:, kt, h, :], pts[(qc, mh)][:, par, kt, :],
                start=(kt == 0), stop=(kt == KT - 1),
            )
            if kt == KT - 1:
                p0 = par * DH
                # NOTE: reciprocal_approx_fast must read SBUF — feeding it
                # PSUM passes CoreSim but returns garbage on hardware.
                rs = rc_pool.tile([DH, CH], f32, tag="rs")
                rc = rc_pool.tile([DH, CH], f32, tag="rc")
                if scalar_copy:
                    # tail-only: exp stream is over, scalar engine is idle
                    nc.scalar.copy(rs, po[DH:P, :])
                else:
                    nc.vector.tensor_copy(rs, po[DH:P, :])
                nc.vector.reciprocal_approx_fast(rc, rs)
                nc.vector.tensor_mul(
                    OT_sb[p0:p0 + DH, mh, qc, :], po[0:DH, :], rc
                )
        return emit

    def outproj_unit(qc, t4, n2, final=False):
        tt = qc * NCH + t4
        ob = o_stage.tile([P, CH], bf16, tag="ob")
        ps = ps_proj.tile([P, CH], f32, tag="proj")
        for m in range(MT):
            nc.tensor.matmul(
                ps,
                OT_sb[:, m, qc, t4 * P:(t4 + 1) * P],
                wo_sb[:, m, n2 * CH:(n2 + 1) * CH],
                start=(m == 0), stop=(m == MT - 1),
            )
        if final and n2 == 0:
            # exp stream is over by the last chunk: use the idle scalar
            # engine for half the casts so the tail's psum->sbuf drain
            # isn't serialized on vector.
            nc.scalar.copy(ob, ps)
        else:
            nc.vector.tensor_copy(ob, ps)
        # stream each half as soon as its cast lands; alternate queues so
        # the final drain parallelizes
        eng = nc.gpsimd if (2 * tt + n2) % 2 == 0 else nc.sync
        eng.dma_start(
            out=out[tt * P:(tt + 1) * P, n2 * CH:(n2 + 1) * CH], in_=ob
        )

    # ---- emission schedule -------------------------------------------
    # lead-in: the sync queue bursts ~300GB/s while the scalar queue only
    # manages ~90GB/s with a ~4us later start — so the head-critical
    # w_k/xk0/xq0 ride sync (in that order), and w_q/w_v overlap on the
    # scalar queue (idle until the first exp).
    w_k = load_w(wkT)
    xk = [load_x(xkT, 0)]
    w_q = load_w(wqT, engine=nc.scalar)
    xq0 = load_x(xqT, 0)
    w_v = load_w(wvT, engine=nc.scalar)
    xk += [load_x(xkT, c) for c in range(1, NCH)]
    xv = [load_x(xvT, c) for c in range(NCH)]
    xq = {1: load_x(xqT, 1)}
    nc.sync.dma_start(out=wo_sb, in_=woT)
    xq.update((c, load_x(xqT, c)) for c in range(2, NCH))
    # V ones-columns fill on the vector engine (x/w rides other queues)
    nc.vector.memset(V_sb[:, :, :, DH:P], 1.0)

    # Background PE work queue: (id, cost_in_512col_slots, emit_fn). The
    # ACT-paced score stream drains it with a per-unit slot budget so the
    # exp stream never sees a multi-us run of foreground-ordered matmuls.
    # Items a stream (or its trailers) DEPENDS on are force-flushed via
    # `needs`/`pre` markers — emitting a reader before its producer would
    # silently order the read first.
    bgq = []
    credit = [0.0]
    bg_id = [0]

    def bg(cost, fn, *args):
        bg_id[0] += 1
        bgq.append((bg_id[0], cost, fn, args))
        return bg_id[0]

    def bg_flush(upto):
        while bgq and bgq[0][0] <= upto:
            _, cost, fn, args = bgq.pop(0)
            fn(*args)
            credit[0] -= cost

    def bg_drain(budget):
        credit[0] = min(credit[0] + budget, 10.0)
        while bgq and credit[0] >= bgq[0][1] * 0.5:
            _, cost, fn, args = bgq.pop(0)
            fn(*args)
            credit[0] -= cost

    pts = {}

    def stream(qc, mh, trailers=(), lag=2, budget=2.3, pre=0, needs=(),
               drain_need=0):
        """Emit one ACT-paced score stream with trailing consumers.

        Per k-tile: the co-issued score pair + its exp ACT, then the
        trailing attn.V chain matmuls (lag k-tiles behind, their ACTs
        long done), then background work up to `budget` PE slots."""
        pt = new_pt()
        pts[(qc, mh)] = pt
        needs = dict(needs)
        bg_flush(pre)
        for kt in range(KT):
            if kt in needs:
                bg_flush(needs[kt])
            scores_unit(qc, mh, pt, kt)
            for tr in trailers:
                tr(kt - lag)
            bg_drain(budget)
        bg_flush(drain_need)
        for kt in range(KT - lag, KT):
            for tr in trailers:
                tr(kt)

    # head: first m-halves only, so the exp stream starts ~4us sooner
    proj_qk_m(w_k, xk[0], 0, 0, is_q=False)
    proj_qk_m(w_q, xq0, 0, 0, is_q=True)

    # stream (0,0): background = remaining K projections, Q0/K m1 halves
    # (prereqs of stream (0,1)), V0/V1 (prereqs of its trailers)
    k1 = bg(8, proj_qk_m, w_k, xk[1], 1, 0, False)
    k2 = bg(8, proj_qk_m, w_k, xk[2], 2, 0, False)
    k3 = bg(8, proj_qk_m, w_k, xk[3], 3, 0, False)
    for c in range(NCH):
        bg(8, proj_qk_m, w_k, xk[c], c, 1, False)
    bg(8, proj_qk_m, w_q, xq0, 0, 1, True)
    for t4 in range(4):
        bg(4, proj_v_t4, w_v, xv[0], 0, t4)
    a00 = [bg(4, proj_v_t4, w_v, xv[1], 1, t4) for t4 in range(4)][-1]
    stream(0, 0, budget=4.3, needs={4: k1, 8: k2, 12: k3})

    # Trailer cascade: stream s carries the attn.V chains of stream s-1
    # (their exps are complete, so the chains are free-running PE work
    # that never stalls the in-order queue), keeping the 2-slot attn.V
    # psum rotation clean: each alloc reuses a slot whose DVE finished
    # one stream ago.
    # stream (0,1): trailers av(0,0/1); background = V2/V3 (flushed just
    # ahead of the trailer k-tiles that read them) + Q1 m0
    v2 = [bg(4, proj_v_t4, w_v, xv[2], 2, t4) for t4 in range(4)]
    v3 = [bg(4, proj_v_t4, w_v, xv[3], 3, t4) for t4 in range(4)]
    q1m0 = bg(8, proj_qk_m, w_q, xq[1], 1, 0, True)
    stream(0, 1, trailers=(make_av(0, 0), make_av(0, 1)), pre=a00,
           needs={10: v2[0], 11: v2[1], 12: v2[2], 13: v2[3],
                  14: v3[0], 15: v3[1]},
           drain_need=v3[3])

    q1m1 = bg(8, proj_qk_m, w_q, xq[1], 1, 1, True)
    stream(1, 0, trailers=(make_av(0, 2), make_av(0, 3)), pre=q1m0)

    for t4 in range(4):
        bg(2, outproj_unit, 0, t4, 0)
        bg(2, outproj_unit, 0, t4, 1)
    q2m0 = bg(8, proj_qk_m, w_q, xq[2], 2, 0, True)
    stream(1, 1, trailers=(make_av(1, 0), make_av(1, 1)), pre=q1m1)

    q2m1 = bg(8, proj_qk_m, w_q, xq[2], 2, 1, True)
    stream(2, 0, trailers=(make_av(1, 2), make_av(1, 3)), pre=q2m0)

    for t4 in range(4):
        bg(2, outproj_unit, 1, t4, 0)
        bg(2, outproj_unit, 1, t4, 1)
    q3m0 = bg(8, proj_qk_m, w_q, xq[3], 3, 0, True)
    stream(2, 1, trailers=(make_av(2, 0), make_av(2, 1)), pre=q2m1)

    q3m1 = bg(8, proj_qk_m, w_q, xq[3], 3, 1, True)
    stream(3, 0, trailers=(make_av(2, 2), make_av(2, 3)), pre=q3m0)

    for t4 in range(4):
        bg(2, outproj_unit, 2, t4, 0)
        bg(2, outproj_unit, 2, t4, 1)
    stream(3, 1, trailers=(make_av(3, 0), make_av(3, 1)), pre=q3m1)

    # tail: leftover background, then the last stream's attn.V chains —
    # av(3,2) runs fully first so its softmax-divide DVE ops overlap
    # av(3,3)'s matmuls instead of serializing after them
    bg_flush(bg_id[0])
    av32 = make_av(3, 2, scalar_copy=True)
    av33 = make_av(3, 3, scalar_copy=True)
    for kt in range(KT):
        av32(kt)
    for kt in range(KT):
        av33(kt)
    for t4 in range(4):
        outproj_unit(3, t4, 0, final=True)
        outproj_unit(3, t4, 1, final=True)


def _build():
    nc = bacc.Bacc("TRN2", target_bir_lowering=False, debug=False)
    xqT = nc.dram_tensor("xqT", [P, NCH, FT, CH], bf16, kind="ExternalInput").ap()
    xkT = nc.dram_tensor("xkT", [P, NCH, FT, CH], bf16, kind="ExternalInput").ap()
    xvT = nc.dram_tensor("xvT", [P, NCH, FT, CH], bf16, kind="ExternalInput").ap()
    wqT = nc.dram_tensor("wqT", [P, FT, F], bf16, kind="ExternalInput").ap()
    wkT = nc.dram_tensor("wkT", [P, FT, F], bf16, kind="ExternalInput").ap()
    wvT = nc.dram_tensor("wvT", [P, FT, F], bf16, kind="ExternalInput").ap()
    woT = nc.dram_tensor("woT", [P, MT, D], bf16, kind="ExternalInput").ap()
    bq2 = nc.dram_tensor("bq2", [P, MT], f32, kind="ExternalInput").ap()
    bk2 = nc.dram_tensor("bk2", [P, MT], f32, kind="ExternalInput").ap()
    bv1 = nc.dram_tensor("bv1", [F], f32, kind="ExternalInput").ap()
    out = nc.dram_tensor("out", [S, D], bf16, kind="ExternalOutput").ap()
    from contextlib import ExitStack

    with tile.TileContext(nc) as tc, ExitStack() as ctx:
        _emit(ctx, nc, tc,
              (xqT, xkT, xvT, wqT, wkT, wvT, woT, bq2, bk2, bv1, out))
    nc.compile()
    nc.m = get_hw_module(nc.m)
    return nc


_cached_nc = None


def _get_nc():
    global _cached_nc
    if _cached_nc is None:
        _cached_nc = _build()
    return _cached_nc


def make_in_maps(query, key, value, Wq, bq, Wk, bk, Wv, bv, Wo, bo):
    query, key, value, Wq, bq, Wk, bk, Wv, bv, Wo = (
        np.asarray(a, np.float32)
        for a in (query, key, value, Wq, bq, Wk, bk, Wv, bv, Wo)
    )
    bf = ml_dtypes.bfloat16

    def pack_x(x):  # [S, D] -> [P, NCH, FT, CH]
        return np.ascontiguousarray(
            x.reshape(NCH, CH, FT, P).transpose(3, 0, 2, 1)).astype(bf)

    def pack_w(W):  # [F, D] -> [P, FT, F]
        return np.ascontiguousarray(
            W.T.reshape(FT, P, F).transpose(1, 0, 2)).astype(bf)

    xTs = [
        tuple(pack_x(a[b]) for a in (query, key, value))
        for b in range(B)
    ]
    in_maps = []
    for c in range(N_CORES):
        b, g = divmod(c, 4)
        fs = slice(g * F, (g + 1) * F)
        qT, kT, vT = xTs[b]
        in_maps.append({
            "xqT": qT,
            "xkT": kT,
            "xvT": vT,
            "wqT": pack_w(Wq[fs]),
            "wkT": pack_w(Wk[fs]),
            "wvT": pack_w(Wv[fs]),
            "woT": np.ascontiguousarray(
                Wo[:, fs].T.reshape(MT, P, D).transpose(1, 0, 2)).astype(bf),
            "bq2": np.ascontiguousarray(bq[fs].reshape(MT, P).T),
            "bk2": np.ascontiguousarray(bk[fs].reshape(MT, P).T),
            "bv1": np.ascontiguousarray(bv[fs]),
        })
    return in_maps


def combine_outputs(core_outs, bo):
    bo = np.asarray(bo, np.float32)
    out = np.empty((B, S, D), np.float32)
    for b in range(B):
        acc = core_outs[4 * b].astype(np.float32)
        for g in range(1, 4):
            acc = acc + core_outs[4 * b + g].astype(np.float32)
        out[b] = acc + bo
    return out


def kernel(query, key, value, Wq, bq, Wk, bk, Wv, bv, Wo, bo, **run_kwargs):
    nc = _get_nc()
    in_maps = make_in_maps(query, key, value, Wq, bq, Wk, bk, Wv, bv, Wo, bo)
    res = run_bass_kernel_spmd(
        nc, in_maps, core_ids=list(range(N_CORES)), **run_kwargs
    )
    out = combine_outputs([r["out"] for r in res.results], bo)
    if run_kwargs:
        kernel.last_results = res
    return out

